# revision 45
# baseline (speedup 1.0000x reference)
"""Trainium2 Bass kernel for nn_CriticNet: obs-proj MLP + single GRU step +
per-agent value heads with routing.

Sharding: pure data-parallel over batch B across 8 NeuronCores; weights
replicated.  Activations are feature-major ([feature partitions, batch free])
so contractions line up and biases are per-partition ACT vectors.  Host prep
re-lays-out tensors so every DMA is one large per-partition-contiguous
transfer.

Matmul operands are bf16 (1 cycle/row on the PE vs 4 for fp32); PSUM
accumulation and all gate math are fp32.  The heads+output stages of chunk i
are emitted during chunks i+1/i+2 so the GRU gate math (ACT/DVE) hides under
the next chunk's PE work and the PE never idles (HAM stays warm).
End-to-end max relative error vs the fp32 reference: ~3.4e-3.
"""

import numpy as np
import ml_dtypes

# ---- problem constants (hardcoded; kernel.py must be self-contained) ----
B, G, H, A = 32768, 2048, 512, 8
PH, HH = 64, 32          # proj hidden, head hidden
NCORES = 8
BL = B // NCORES         # 4096 batch rows per core
NT = 512                 # batch tile (matmul moving free dim)
NB = BL // NT            # 8 batch chunks per core
KG = G // 128            # 16 k-chunks for obs
KH = H // 128            # 4 k-chunks for H-sized contractions

BF = ml_dtypes.bfloat16
F32 = np.float32

_compiled = None  # cached compiled program — compile once per process
last_run = None   # BassKernelResults of the most recent kernel() call


def _build_program():
    import concourse.bacc as bacc
    import concourse.tile as tile
    from concourse import mybir

    dt = mybir.dt
    AF = mybir.ActivationFunctionType
    ALU = mybir.AluOpType

    nc = bacc.Bacc("TRN2", target_bir_lowering=False, debug=False)

    def din(name, shape, dtype):
        return nc.dram_tensor(name, shape, dtype, kind="ExternalInput").ap()

    def dout(name, shape, dtype):
        return nc.dram_tensor(name, shape, dtype, kind="ExternalOutput").ap()

    # inputs (per core)
    obsT = din("obsT", [NB, 128, KG, NT], dt.bfloat16)
    hpb = din("hpb", [NB, 128, KH, NT], dt.bfloat16)
    hpf = din("hpf", [NB, 128, KH, NT], dt.float32)
    mskT = din("mskT", [NB, 128, 2, NT], dt.bfloat16)
    oneh = din("oneh", [8, NB, NT], dt.bfloat16)
    w0 = din("w0", [128, KG, PH], dt.bfloat16)
    w1 = din("w1", [PH, H], dt.bfloat16)
    wihT = din("wihT", [128, KH, 3 * H], dt.bfloat16)
    whhT = din("whhT", [128, KH, 3 * H], dt.bfloat16)
    wh1 = din("wh1", [128, KH, 2 * 128], dt.bfloat16)
    w2f = din("w2f", [128, 2, 1], dt.bfloat16)
    b2v = din("b2v", [8, 1], dt.bfloat16)
    b0 = din("b0", [PH, 1], dt.float32)
    b1 = din("b1", [128, KH], dt.float32)
    brz = din("brz", [128, 8], dt.float32)
    bihn = din("bihn", [128, KH], dt.float32)
    bhhn = din("bhhn", [128, KH], dt.float32)
    bh1 = din("bh1", [128, 2], dt.float32)

    # outputs (per core)
    outv = dout("outv", [NB, 1, NT], dt.float32)
    hnxT = dout("hnxT", [NB, 128, KH, NT], dt.float32)
    # throwaway sink for the PE warm-up matmuls (keeps them from being DCE'd)
    warm = dout("warm", [128, NT], dt.float32)

    with tile.TileContext(nc) as tc:
        with (
            tc.tile_pool(name="const", bufs=1) as const,
            tc.tile_pool(name="obs", bufs=2) as obs_pool,
            tc.tile_pool(name="hp", bufs=2) as hp_pool,
            tc.tile_pool(name="msk", bufs=2) as msk_pool,
            tc.tile_pool(name="xio", bufs=2) as xio_pool,
            tc.tile_pool(name="gates", bufs=2) as gates_pool,
            tc.tile_pool(name="hout", bufs=2) as hout_pool,
            tc.tile_pool(name="tmp", bufs=3) as tmp_pool,
            tc.tile_pool(name="mix", bufs=1) as mix_pool,
            tc.tile_pool(name="outp", bufs=2) as out_pool,
            tc.tile_pool(name="ps", bufs=8, space="PSUM") as ps,
        ):
            # ---- PE warm-up: dense dummy matmuls on a memset tile while the
            # first DMAs are still in flight (first DMA byte lands ~8us in),
            # so HAM is at 8/8 clock when the real L1 matmuls start ----
            dj = tmp_pool.tile([128, NT], dt.bfloat16, tag="dj")
            nc.vector.memset(dj[:], 0.0)
            pdum = ps.tile([128, NT], dt.float32, tag="ps")
            N_WARM = 14
            for i in range(N_WARM):
                nc.tensor.matmul(
                    pdum[:], dj[:, 0:128], dj[:],
                    start=(i == 0), stop=(i == N_WARM - 1),
                )
            wsink = tmp_pool.tile([128, NT], dt.float32, tag="wsink")
            nc.scalar.activation(wsink[:], pdum[:], AF.Copy)
            nc.sync.dma_start(warm[:], wsink[:])

            # ---- DMAs ordered by first use: L1 deps, chunk-0 obs (split so
            # the first matmul is gated on 128KB), GRU weights, then the
            # late-consumed tensors ----
            w0_sb = const.tile([128, KG, PH], dt.bfloat16)
            nc.sync.dma_start(w0_sb[:], w0[:])
            b0_sb = const.tile([PH, 1], dt.float32)
            nc.sync.dma_start(b0_sb[:], b0[:])

            def load_inputs(bc):
                obs_sb = obs_pool.tile([128, KG, NT], dt.bfloat16, tag="obs")
                if bc == 0:
                    nc.sync.dma_start(obs_sb[:, 0:1, :], obsT[bc, :, 0:1, :])
                    nc.sync.dma_start(obs_sb[:, 1:2, :], obsT[bc, :, 1:2, :])
                    nc.sync.dma_start(obs_sb[:, 2:, :], obsT[bc, :, 2:, :])
                else:
                    nc.sync.dma_start(obs_sb[:], obsT[bc])
                hpb_sb = hp_pool.tile([128, KH, NT], dt.bfloat16, tag="hpb")
                nc.sync.dma_start(hpb_sb[:], hpb[bc])
                hpf_sb = hp_pool.tile([128, KH, NT], dt.float32, tag="hpf")
                nc.sync.dma_start(hpf_sb[:], hpf[bc])
                msk_sb = msk_pool.tile([128, 2, NT], dt.bfloat16, tag="msk")
                nc.sync.dma_start(msk_sb[:], mskT[bc])
                return obs_sb, hpb_sb, hpf_sb, msk_sb

            # chunk-0: obs + h_prev(bf16) first; defer hpf/msk to after the
            # GRU weights (they are consumed much later)
            obs_sb0 = obs_pool.tile([128, KG, NT], dt.bfloat16, tag="obs")
            nc.sync.dma_start(obs_sb0[:, 0:1, :], obsT[0, :, 0:1, :])
            nc.sync.dma_start(obs_sb0[:, 1:2, :], obsT[0, :, 1:2, :])
            nc.sync.dma_start(obs_sb0[:, 2:, :], obsT[0, :, 2:, :])
            hpb_sb0 = hp_pool.tile([128, KH, NT], dt.bfloat16, tag="hpb")
            nc.sync.dma_start(hpb_sb0[:], hpb[0])

            w1_sb = const.tile([PH, H], dt.bfloat16)
            nc.sync.dma_start(w1_sb[:], w1[:])
            b1_sb = const.tile([128, KH], dt.float32)
            nc.sync.dma_start(b1_sb[:], b1[:])
            wih_sb = const.tile([128, KH, 3 * H], dt.bfloat16)
            nc.sync.dma_start(wih_sb[:], wihT[:])
            brz_sb = const.tile([128, 8], dt.float32)
            nc.sync.dma_start(brz_sb[:], brz[:])
            whh_sb = const.tile([128, KH, 3 * H], dt.bfloat16)
            nc.sync.dma_start(whh_sb[:], whhT[:])
            bihn_sb = const.tile([128, KH], dt.float32)
            nc.sync.dma_start(bihn_sb[:], bihn[:])
            bhhn_sb = const.tile([128, KH], dt.float32)
            nc.sync.dma_start(bhhn_sb[:], bhhn[:])

            hpf_sb0 = hp_pool.tile([128, KH, NT], dt.float32, tag="hpf")
            nc.sync.dma_start(hpf_sb0[:], hpf[0])
            msk_sb0 = msk_pool.tile([128, 2, NT], dt.bfloat16, tag="msk")
            nc.sync.dma_start(msk_sb0[:], mskT[0])
            ins0 = (obs_sb0, hpb_sb0, hpf_sb0, msk_sb0)

            wh1_sb = const.tile([128, KH, 2 * 128], dt.bfloat16)
            nc.sync.dma_start(wh1_sb[:], wh1[:])
            bh1_sb = const.tile([128, 2], dt.float32)
            nc.sync.dma_start(bh1_sb[:], bh1[:])
            w2f_sb = const.tile([128, 2, 1], dt.bfloat16)
            nc.sync.dma_start(w2f_sb[:], w2f[:])
            b2v_sb = const.tile([8, 1], dt.bfloat16)
            nc.sync.dma_start(b2v_sb[:], b2v[:])
            oneh_sb = const.tile([8, NB, NT], dt.bfloat16)
            nc.sync.dma_start(oneh_sb[:], oneh[:])

            # per-chunk state carried across pipeline stages
            state = {}

            def mm_and_gates(bc, ins):
                """MLP + GRU matmuls and gate math for chunk bc."""
                obs_sb, hpb_sb, hpf_sb, msk_sb = ins

                # obs proj layer 1: t = relu(obs @ W0 + b0)
                pt = ps.tile([PH, NT], dt.float32, tag="ps")
                for kc in range(KG):
                    nc.tensor.matmul(
                        pt[:], w0_sb[:, kc, :], obs_sb[:, kc, :],
                        start=(kc == 0), stop=(kc == KG - 1),
                    )
                t_bf = xio_pool.tile([PH, NT], dt.bfloat16, tag="t")
                nc.scalar.activation(t_bf[:], pt[:], AF.Relu, bias=b0_sb[:])

                # layer 2: x = t @ W1 + b1
                x_bf = xio_pool.tile([128, KH, NT], dt.bfloat16, tag="x")
                for mc in range(KH):
                    px = ps.tile([128, NT], dt.float32, tag="ps")
                    nc.tensor.matmul(
                        px[:], w1_sb[:, mc * 128:(mc + 1) * 128], t_bf[:],
                        start=True, stop=True,
                    )
                    nc.scalar.activation(
                        x_bf[:, mc, :], px[:], AF.Identity,
                        bias=b1_sb[:, mc:mc + 1],
                    )

                # GRU r,z gates
                r_f = gates_pool.tile([128, KH, NT], dt.float32, tag="r", bufs=1)
                z_f = gates_pool.tile([128, KH, NT], dt.float32, tag="z")
                for gc in range(8):
                    prz = ps.tile([128, NT], dt.float32, tag="ps")
                    for kk in range(KH):
                        nc.tensor.matmul(
                            prz[:], wih_sb[:, kk, gc * 128:(gc + 1) * 128],
                            x_bf[:, kk, :], start=(kk == 0), stop=False,
                        )
                    for kk in range(KH):
                        nc.tensor.matmul(
                            prz[:], whh_sb[:, kk, gc * 128:(gc + 1) * 128],
                            hpb_sb[:, kk, :], start=False, stop=(kk == KH - 1),
                        )
                    dst = r_f[:, gc, :] if gc < KH else z_f[:, gc - KH, :]
                    nc.scalar.activation(
                        dst, prz[:], AF.Sigmoid, bias=brz_sb[:, gc:gc + 1]
                    )

                # GRU n gate: tanh(i_n + bihn + r * (hh_n + bhhn))
                n_f = gates_pool.tile([128, KH, NT], dt.float32, tag="n")
                for kc in range(KH):
                    gc = 8 + kc
                    pin = ps.tile([128, NT], dt.float32, tag="ps")
                    for kk in range(KH):
                        nc.tensor.matmul(
                            pin[:], wih_sb[:, kk, gc * 128:(gc + 1) * 128],
                            x_bf[:, kk, :], start=(kk == 0), stop=(kk == KH - 1),
                        )
                    phn = ps.tile([128, NT], dt.float32, tag="ps")
                    for kk in range(KH):
                        nc.tensor.matmul(
                            phn[:], whh_sb[:, kk, gc * 128:(gc + 1) * 128],
                            hpb_sb[:, kk, :], start=(kk == 0), stop=(kk == KH - 1),
                        )
                    rhn = tmp_pool.tile([128, NT], dt.float32, tag="tmp")
                    nc.vector.scalar_tensor_tensor(
                        rhn[:], phn[:], bhhn_sb[:, kc:kc + 1], r_f[:, kc, :],
                        ALU.add, ALU.mult,
                    )
                    s = tmp_pool.tile([128, NT], dt.float32, tag="tmp")
                    nc.vector.tensor_add(s[:], pin[:], rhn[:])
                    nc.scalar.activation(
                        n_f[:, kc, :], s[:], AF.Tanh, bias=bihn_sb[:, kc:kc + 1]
                    )
                state[bc] = dict(n_f=n_f, z_f=z_f, hpf=hpf_sb, msk=msk_sb)

            def h_mix(bc, narrow=False):
                """h = n + z*(hprev - n); batched wide unless `narrow` (the
                last chunk uses per-kc ops so the tail heads matmuls can
                start on each kc slice as soon as it is ready)."""
                st = state[bc]
                n_f, z_f, hpf_sb = st["n_f"], st["z_f"], st["hpf"]
                h_f = hout_pool.tile([128, KH, NT], dt.float32, tag="hf")
                h_bf = hout_pool.tile([128, KH, NT], dt.bfloat16, tag="hb")
                if narrow:
                    for kc in range(KH):
                        d = mix_pool.tile([128, NT], dt.float32, tag="dn")
                        nc.vector.tensor_sub(d[:], hpf_sb[:, kc, :], n_f[:, kc, :])
                        nc.vector.tensor_mul(d[:], z_f[:, kc, :], d[:])
                        nc.vector.tensor_add(h_f[:, kc, :], n_f[:, kc, :], d[:])
                        nc.vector.tensor_copy(h_bf[:, kc, :], h_f[:, kc, :])
                else:
                    nv = n_f[:].rearrange("p k j -> p (k j)")
                    d = mix_pool.tile([128, KH * NT], dt.float32, tag="d")
                    nc.vector.tensor_sub(
                        d[:], hpf_sb[:].rearrange("p k j -> p (k j)"), nv)
                    nc.vector.tensor_mul(
                        d[:], z_f[:].rearrange("p k j -> p (k j)"), d[:])
                    nc.vector.tensor_add(
                        h_f[:].rearrange("p k j -> p (k j)"), nv, d[:])
                    nc.vector.tensor_copy(
                        h_bf[:].rearrange("p k j -> p (k j)"),
                        h_f[:].rearrange("p k j -> p (k j)"),
                    )
                nc.sync.dma_start(hnxT[bc], h_f[:])
                st["h_bf"] = h_bf

            def heads_front(bc):
                """per-agent head hidden layers + masking for chunk bc."""
                st = state[bc]
                h_bf, msk_sb = st["h_bf"], st["msk"]
                hms = []
                for g in range(2):
                    phd = ps.tile([128, NT], dt.float32, tag="ps")
                    for kk in range(KH):
                        nc.tensor.matmul(
                            phd[:], wh1_sb[:, kk, g * 128:(g + 1) * 128],
                            h_bf[:, kk, :],
                            start=(kk == 0), stop=(kk == KH - 1),
                        )
                    hid = tmp_pool.tile([128, NT], dt.bfloat16, tag="hid")
                    nc.scalar.activation(
                        hid[:], phd[:], AF.Relu, bias=bh1_sb[:, g:g + 1]
                    )
                    hm = tmp_pool.tile([128, NT], dt.bfloat16, tag="hm")
                    nc.vector.tensor_mul(hm[:], hid[:], msk_sb[:, g, :])
                    hms.append(hm)
                st["hms"] = hms

            def po_out(bc):
                """final per-agent dot + bias-by-agent + store for chunk bc."""
                st = state.pop(bc)
                hms = st["hms"]
                po = ps.tile([1, NT], dt.float32, tag="ps")
                for g in range(2):
                    nc.tensor.matmul(
                        po[:], w2f_sb[:, g, :], hms[g][:],
                        start=(g == 0), stop=False,
                    )
                nc.tensor.matmul(
                    po[:], b2v_sb[:], oneh_sb[:, bc, :], start=False, stop=True,
                )
                o_sb = out_pool.tile([1, NT], dt.float32, tag="o")
                nc.scalar.activation(o_sb[:], po[:], AF.Copy)
                nc.sync.dma_start(outv[bc], o_sb[:])

            # ---- software-pipelined main loop ----
            for bc in range(NB):
                ins = ins0 if bc == 0 else load_inputs(bc)
                mm_and_gates(bc, ins)
                if bc >= 1:
                    heads_front(bc - 1)
                h_mix(bc, narrow=(bc == NB - 1))
                if bc >= 2:
                    po_out(bc - 2)
            heads_front(NB - 1)
            po_out(NB - 2)
            po_out(NB - 1)

    nc.compile()
    return nc


def _prep_core_inputs(inputs, c):
    """Host-side re-layout of the batch shard for core c (weights shared)."""
    sl = slice(c * BL, (c + 1) * BL)
    obs = inputs["global_obs"][sl]                     # [BL, G] f32
    hp = inputs["h_critic_prev"][0, sl]                # [BL, H] f32
    ids = np.asarray(inputs["agent_ids"][sl]).astype(np.int64)  # [BL]

    obsT = np.ascontiguousarray(
        obs.astype(BF).reshape(NB, NT, KG, 128).transpose(0, 3, 2, 1)
    )
    hp_blk_f = np.ascontiguousarray(
        hp.reshape(NB, NT, KH, 128).transpose(0, 3, 2, 1)
    )
    hp_blk_b = np.ascontiguousarray(hp_blk_f.astype(BF))
    # mask[bc, p, g, j] = (ids[bc*NT+j] == g*4 + p//HH)
    idsb = ids.reshape(NB, 1, 1, NT)
    agent_of_row = (np.arange(2 * 128) // HH).reshape(2, 128).transpose(1, 0)
    mskT = (idsb == agent_of_row.reshape(1, 128, 2, 1)).astype(BF)
    oneh = (ids.reshape(1, NB, NT) == np.arange(A).reshape(A, 1, 1)).astype(BF)
    return {
        "obsT": obsT,
        "hpb": hp_blk_b,
        "hpf": np.ascontiguousarray(hp_blk_f, dtype=F32),
        "mskT": np.ascontiguousarray(mskT),
        "oneh": np.ascontiguousarray(oneh),
    }


def _prep_weights(inputs):
    w0 = inputs["proj_w0"].astype(BF)                  # [G, PH]
    w1 = inputs["proj_w1"].astype(BF)                  # [PH, H]
    wih = inputs["gru_w_ih"].astype(F32)               # [3H, H]
    whh = inputs["gru_w_hh"].astype(F32)
    hw1 = inputs["head_w1"].astype(F32)                # [A, H, HH]
    hw2 = inputs["head_w2"].astype(F32)                # [A, HH]
    b_ih = inputs["gru_b_ih"].astype(F32)
    b_hh = inputs["gru_b_hh"].astype(F32)

    out = {}
    out["w0"] = np.ascontiguousarray(
        w0.reshape(KG, 128, PH).transpose(1, 0, 2)
    )
    out["w1"] = np.ascontiguousarray(w1)
    out["wihT"] = np.ascontiguousarray(
        wih.T.reshape(KH, 128, 3 * H).transpose(1, 0, 2).astype(BF)
    )
    out["whhT"] = np.ascontiguousarray(
        whh.T.reshape(KH, 128, 3 * H).transpose(1, 0, 2).astype(BF)
    )
    # wh1[p, kk, a*HH+j] = head_w1[a, kk*128+p, j]
    out["wh1"] = np.ascontiguousarray(
        hw1.transpose(1, 0, 2).reshape(H, A * HH)
        .reshape(KH, 128, A * HH).transpose(1, 0, 2).astype(BF)
    )
    # w2f[a'*HH+j, g, 0] = head_w2[g*4+a', j]
    out["w2f"] = np.ascontiguousarray(
        hw2.reshape(2, 4 * HH).transpose(1, 0).reshape(128, 2, 1).astype(BF)
    )
    out["b2v"] = np.ascontiguousarray(inputs["head_b2"].astype(BF).reshape(8, 1))
    out["b0"] = np.ascontiguousarray(inputs["proj_b0"].astype(F32).reshape(PH, 1))
    out["b1"] = np.ascontiguousarray(
        inputs["proj_b1"].astype(F32).reshape(KH, 128).T
    )
    out["brz"] = np.ascontiguousarray(
        (b_ih + b_hh)[: 2 * H].reshape(8, 128).T
    )
    out["bihn"] = np.ascontiguousarray(b_ih[2 * H:].reshape(KH, 128).T)
    out["bhhn"] = np.ascontiguousarray(b_hh[2 * H:].reshape(KH, 128).T)
    # bh1[a'*HH+j, g] = head_b1[g*4+a', j]
    out["bh1"] = np.ascontiguousarray(
        inputs["head_b1"].astype(F32).reshape(2, 4 * HH).T
    )
    return out


def kernel(**inputs):
    global _compiled, last_run
    from concourse.bass_utils import run_bass_kernel_spmd

    if _compiled is None:
        _compiled = _build_program()
    nc = _compiled

    weights = _prep_weights(inputs)
    in_maps = []
    for c in range(NCORES):
        m = dict(weights)
        m.update(_prep_core_inputs(inputs, c))
        in_maps.append(m)

    br = run_bass_kernel_spmd(nc, in_maps, core_ids=list(range(NCORES)))
    last_run = br

    out = np.empty((B,), dtype=F32)
    hnext = np.empty((1, B, H), dtype=F32)
    for c in range(NCORES):
        res = br.results[c]
        sl = slice(c * BL, (c + 1) * BL)
        out[sl] = res["outv"].reshape(BL)
        # hnxT[bc, p, kc, j] -> h[bc*NT+j, kc*128+p]
        hnext[0, sl] = res["hnxT"].transpose(0, 3, 2, 1).reshape(BL, H)
    return out, hnext
